# revision 21
# baseline (speedup 1.0000x reference)
"""BatchedGraphSAGEMean1Temporal Trainium2 kernel.

Strategy (8 NeuronCores, data-parallel over batch B=8, one graph/core):

The three neighbor-mean gathers share indices across the batch, so each
gather+mean is a fixed linear operator A_k [N,N] (A_k[n,j] = multiplicity
of j among n's 32 neighbors, / 32).  Since the op is linear,
    h_k = neib_mean_k(x) @ Wn^T + bn = A_k @ (x @ Wn^T) + bn,
so Wn is applied once and the three gathers become three dense matmuls
with host-prebuilt A_k (exact in bf16: entries are small-int/32).

Per-core pipeline:
  - PE-transpose x -> xT, then [h_self | y] = [x|1] @ [[WxT|WnT];[bx|bn]]
    as fp32r matmuls (full PE rate).
  - h_k = A_cat @ y + bn as bf16 matmuls (A exact, y rounded to bf16).
  - L2 norm: ACT Square with accum_out -> ss, inv = 1/sqrt(ss),
    g = max(inv*h, 0) fused on DVE.
  - BatchNorm stats: ones-column matmuls give per-channel sum/sumsq rows;
    AllGather(8KB) across the 8 cores; reduce + scale/shift math in a
    [128,8] layout; broadcast s,t via K=1 matmuls; out = g*S + T.
"""

import os
import sys

if "/opt/trn_rl_repo" not in sys.path:
    sys.path.insert(0, "/opt/trn_rl_repo")

import numpy as np
import ml_dtypes

import concourse.bass as bass
import concourse.mybir as mybir
import concourse.tile as tile
from concourse.bass_utils import run_bass_kernel_spmd
from concourse.vector_clock import ScopedClock

BF16 = mybir.dt.bfloat16
F32 = mybir.dt.float32
F32R = mybir.dt.float32r
AF = mybir.ActivationFunctionType
ALU = mybir.AluOpType

N_CORES = 8
B, N, F, C = 8, 512, 256, 1024
NMAX = 32
NCH = N // 128          # 4 node chunks
FCH = F // 128          # 2 feature chunks
NSETS = 3
BN_EPS = 1e-5
WARMUP_MMS = 8


# ---------------------------------------------------------------------------
# Workaround: walrus rejects >2 sync waits on one ctrl instruction; Tile's
# kernel-tail drain carries one wait per completion sem.  Spread them over
# nofuse NOPs (SP executes serially, so wait-then-drain is equivalent).
def _spread_drain_and_barrier(self, tick_clock, wait_clock):
    nc = self.nc
    probe = nc.sync.nop(nofuse=True)
    wait_clock.add_sem_waits(probe.ins, ScopedClock({None: tick_clock.global_clock}))
    si = probe.ins.sync_info
    waits = list(si.on_wait or []) if si is not None else []
    if si is not None:
        probe.ins.sync_info = mybir.SyncInfo(on_wait=waits[:1], on_update=si.on_update)
    for w in waits[1:]:
        n = nc.sync.nop(nofuse=True)
        n.ins.sync_info = mybir.SyncInfo(on_wait=[w], on_update=[])
    nc.sync.drain()
    nc.all_engine_barrier()
    assert self.sems is not None
    popped = nc._tile_sem_poison_stack.pop()
    assert popped is self._sem_poison
    nc.clear_and_free_semaphores(list(self.sems.allocated().values()))
    nc.all_engine_barrier()


tile.TileContext._drain_and_barrier = _spread_drain_and_barrier


def _spread_waits(nc, limit=1):
    """Move excess per-instruction sem waits onto preceding same-engine NOPs
    (walrus caps sync waits per instruction; engines execute serially so
    wait-then-op is equivalent)."""
    cnt = 0
    for fn in nc.m.functions:
        for bb in fn.blocks:
            il = bb.instructions
            new = []
            for inst in il:
                si = getattr(inst, "sync_info", None)
                waits = list(si.on_wait or []) if si is not None else []
                if len(waits) > limit:
                    keep = waits[:limit]
                    extra = waits[limit:]
                    for i in range(0, len(extra), limit):
                        nop = mybir.InstNoOp(name=f"wsplit-{cnt}", ins=[], outs=[])
                        cnt += 1
                        nop.engine = inst.engine
                        nop.sync_info = mybir.SyncInfo(
                            on_wait=extra[i:i + limit], on_update=[]
                        )
                        new.append(nop)
                    inst.sync_info = mybir.SyncInfo(
                        on_wait=keep, on_update=si.on_update
                    )
                new.append(inst)
            if len(new) != len(il):
                il[:] = new
    return cnt
# ---------------------------------------------------------------------------


def _build_module(skip_wb=False, skip_bnb=False):
    nc = bass.Bass("TRN2", target_bir_lowering=False, debug=False,
                   num_devices=N_CORES)

    # ---- DRAM I/O ----
    x_d = nc.dram_tensor("x", [128, NCH, F], F32R, kind="ExternalInput").ap()
    wc_d = nc.dram_tensor("wc", [128, FCH, 2 * F], F32R, kind="ExternalInput").ap()
    wb_d = nc.dram_tensor("wb", [1, 2 * F], F32R, kind="ExternalInput").ap()
    a_d = nc.dram_tensor("a", [NCH, 128, NCH, NSETS * 128], BF16,
                         kind="ExternalInput").ap()
    bnr_d = nc.dram_tensor("bnr", [1, F], BF16, kind="ExternalInput").ap()
    gb_d = nc.dram_tensor("gb", [128, 16], F32, kind="ExternalInput").ap()
    ident_d = nc.dram_tensor("ident", [128, 128], F32R, kind="ExternalInput").ap()
    row32_d = nc.dram_tensor("row32", [1, 128], F32R, kind="ExternalInput").ap()
    col32_d = nc.dram_tensor("col32", [128, 1], F32R, kind="ExternalInput").ap()
    out_d = nc.dram_tensor("out", [N, C], F32, kind="ExternalOutput").ap()

    cc_in_d = nc.dram_tensor("cc_in", [1, 2 * C], F32R)
    cc_out_d = nc.dram_tensor("cc_out", [N_CORES, 2 * C], F32R,
                              addr_space="Shared")

    with tile.TileContext(nc) as tc:
        with (
            tc.tile_pool(name="const", bufs=1) as constp,
            tc.tile_pool(name="inp", bufs=1) as inp,
            tc.tile_pool(name="work", bufs=1) as work,
            tc.tile_pool(name="sq", bufs=3) as sqp,
            tc.tile_pool(name="apply", bufs=2) as applyp,
            tc.tile_pool(name="outp", bufs=2) as outp,
            tc.tile_pool(name="small", bufs=1) as small,
        ):
            # ---- input DMAs: critical tensors first, split across the
            # SP and ACT HWDGE queues ----
            x_sb = inp.tile([128, NCH, F], F32R)
            nc.sync.dma_start(x_sb[:], x_d[:])
            ident = constp.tile([128, 128], F32R)
            nc.sync.dma_start(ident[:], ident_d[:])
            wc_sb = inp.tile([128, FCH, 2 * F], F32R)
            nc.sync.dma_start(wc_sb[:], wc_d[:])
            a_sb = inp.tile([128, NCH * NCH, NSETS * 128], BF16)
            for g in range(NCH):
                deng = nc.sync if g % 2 == 0 else nc.scalar
                deng.dma_start(a_sb[:, g * NCH:(g + 1) * NCH, :], a_d[g])
            wb_sb = inp.tile([1, 2 * F], F32R)
            nc.scalar.dma_start(wb_sb[:], wb_d[:])
            bnr_sb = inp.tile([1, F], BF16)
            nc.scalar.dma_start(bnr_sb[:], bnr_d[:])
            gb_sb = constp.tile([128, 16], F32)     # gamma blocks | beta blocks
            nc.scalar.dma_start(gb_sb[:], gb_d[:])
            row32 = constp.tile([1, 128], F32R)
            nc.scalar.dma_start(row32[:], row32_d[:])
            col32 = constp.tile([128, 1], F32R)
            nc.scalar.dma_start(col32[:], col32_d[:])
            row16 = constp.tile([1, 128], BF16)
            nc.vector.memset(row16[:], 1.0)
            eps5 = constp.tile([128, 1], F32)
            nc.vector.memset(eps5[:], BN_EPS)
            eps24 = constp.tile([128, 1], F32)
            nc.vector.memset(eps24[:], 1e-24)

            wsrc = constp.tile([128, 512], BF16)
            nc.vector.memset(wsrc[:], 0.0)
            wsrc = constp.tile([128, 512], BF16)
            nc.vector.memset(wsrc[:], 0.0)
            xT_sb = work.tile([128, FCH, N], F32R)
            y_bf = work.tile([128, NCH, F], BF16)
            r_bf = work.tile([128, NCH, C], BF16)
            rsq_bf = work.tile([128, NCH, C], BF16)
            g_bf = work.tile([128, NCH, C], BF16)
            ss4 = small.tile([128, 4 * NCH], F32)   # per-piece sumsq accums
            ss = small.tile([128, NCH], F32)
            std = small.tile([128, NCH], F32)
            inv = small.tile([128, NCH], F32)
            inv2 = small.tile([128, NCH], F32)
            invc = small.tile([128, 2 * NCH], BF16)

            def keep_warm(pool, n=1):
                for _ in range(n):
                    pbd = pool.tile([128, F], F32, tag="pb")
                    nc.tensor.matmul(pbd[:], wsrc[:, 0:128], wsrc[:, 0:F],
                                     start=True, stop=True)

            def keep_warm(pool, n=1):
                for _ in range(n):
                    pbd = pool.tile([128, F], F32, tag="pb")
                    nc.tensor.matmul(pbd[:], wsrc[:, 0:128], wsrc[:, 0:F],
                                     start=True, stop=True)

            def evac_pieces(c4, pieces):
                """Fused PSUM evacuation: r = relu(h) (DVE/ACT split),
                sq-accum (ACT), y cast for MM-A.  pieces = [(psum_ap, s)]"""
                for pap, s in pieces:
                    dst = r_bf[:, c4, s * F:(s + 1) * F]
                    if s % 2 == 0:
                        nc.vector.tensor_scalar(dst, pap, 0.0, None, ALU.max)
                    else:
                        nc.scalar.activation(dst, pap, AF.Relu)
                    sqt = sqp.tile([128, F], F32)
                    nc.scalar.activation(
                        sqt[:], pap, AF.Square,
                        accum_out=ss4[:, 4 * c4 + s:4 * c4 + s + 1],
                    )

            def chunk_stats(c4, spsum_tiles):
                """Per-chunk: ss -> inv -> stat-matmul columns -> stat MMs."""
                smu, se2 = spsum_tiles
                nc.vector.tensor_reduce(
                    ss[:, c4:c4 + 1], ss4[:, 4 * c4:4 * (c4 + 1)],
                    axis=mybir.AxisListType.X, op=ALU.add,
                )
                nc.scalar.activation(
                    std[:, c4:c4 + 1], ss[:, c4:c4 + 1], AF.Sqrt, bias=eps24[:])
                nc.vector.reciprocal(inv[:, c4:c4 + 1], std[:, c4:c4 + 1])
                nc.vector.tensor_tensor(
                    inv2[:, c4:c4 + 1], inv[:, c4:c4 + 1], inv[:, c4:c4 + 1],
                    ALU.mult)
                nc.vector.tensor_scalar(
                    invc[:, c4:c4 + 1], inv[:, c4:c4 + 1],
                    1.0 / (B * N), None, ALU.mult)
                nc.vector.tensor_scalar(
                    invc[:, NCH + c4:NCH + c4 + 1], inv2[:, c4:c4 + 1],
                    1.0 / (B * N), None, ALU.mult)
                for half in range(2):
                    sl = slice(half * 512, (half + 1) * 512)
                    nc.tensor.matmul(
                        smu[:, sl], invc[:, c4:c4 + 1], r_bf[:, c4, sl],
                        start=(c4 == 0), stop=(c4 == NCH - 1),
                    )
                    nc.tensor.matmul(
                        se2[:, sl], invc[:, NCH + c4:NCH + c4 + 1],
                        rsq_bf[:, c4, sl],
                        start=(c4 == 0), stop=(c4 == NCH - 1),
                    )

            spsum_cm = tc.tile_pool(name="spsum", bufs=1, space="PSUM")
            spsum = spsum_cm.__enter__()
            with tc.tile_pool(name="tpsum", bufs=2, space="PSUM") as tpsum:
                # ---- PE warmup burst (HAM un-throttle during input DMAs) ----
                for w in range(WARMUP_MMS):
                    wp = tpsum.tile([128, 512], F32, tag="apsum")
                    nc.tensor.matmul(wp[:], wsrc[:, 0:128], wsrc[:],
                                     start=True, stop=True)

                # ---- transpose x (PE) ----
                for c4 in range(NCH):
                    for fc in range(FCH):
                        pt = tpsum.tile([128, 128], F32R)
                        nc.tensor.transpose(
                            pt[:], x_sb[:, c4, fc * 128:(fc + 1) * 128], ident[:]
                        )
                        nc.vector.tensor_copy(
                            xT_sb[:, fc, c4 * 128:(c4 + 1) * 128], pt[:]
                        )

                smu = spsum.tile([1, C], F32)
                se2 = spsum.tile([1, C], F32)

                # ---- MM-A: [h_self | y] = [x|1] @ [[WxT|WnT];[bx|bn]] ----
                pa_tiles = []
                for c4 in range(NCH):
                    pa = tpsum.tile([128, 2 * F], F32, tag="apsum")
                    for kc in range(FCH):
                        nc.tensor.matmul(
                            pa[:],
                            xT_sb[:, kc, c4 * 128:(c4 + 1) * 128],
                            wc_sb[:, kc, :],
                            start=(kc == 0), stop=(skip_wb and kc == FCH - 1),
                        )
                    if not skip_wb:
                        nc.tensor.matmul(
                            pa[:], row32[:], wb_sb[:], start=False, stop=True,
                        )
                    nc.vector.tensor_copy(y_bf[:, c4, :], pa[:, F:2 * F])
                    evac_pieces(c4, [(pa[:, 0:F], 0)])

            # ---- MM-B + fused epilogue, per node chunk ----
            with tc.tile_pool(name="bpsum", bufs=3, space="PSUM") as bpsum:
                keep_warm(bpsum, 4)
                for c4 in range(NCH):
                    for s in range(NSETS):
                        pb = bpsum.tile([128, F], F32, tag="pb")
                        for kc in range(NCH):
                            nc.tensor.matmul(
                                pb[:],
                                a_sb[:, c4 * NCH + kc, s * 128:(s + 1) * 128],
                                y_bf[:, kc, :],
                                start=(kc == 0),
                                stop=(skip_bnb and kc == NCH - 1),
                            )
                        if not skip_bnb:
                            nc.tensor.matmul(
                                pb[:], row16[:], bnr_sb[:],
                                start=False, stop=True,
                            )
                        evac_pieces(c4, [(pb[:], s + 1)])
                    nc.vector.tensor_tensor(
                        rsq_bf[:, c4, :], r_bf[:, c4, :], r_bf[:, c4, :],
                        ALU.mult)
                    chunk_stats(c4, (smu, se2))
                    keep_warm(bpsum, 1)
                    keep_warm(bpsum, 1)

                stat_sb = small.tile([1, 2 * C], F32R)
                nc.vector.tensor_copy(stat_sb[:, 0:C], smu[:])
                nc.scalar.copy(stat_sb[:, C:2 * C], se2[:])
            spsum_cm.__exit__(None, None, None)

            # ---- AllGather local stats across the 8 cores ----
            nc.sync.dma_start(cc_in_d[:], stat_sb[:])
            nc.gpsimd.collective_compute(
                "AllGather", ALU.bypass,
                replica_groups=[list(range(N_CORES))],
                ins=[cc_in_d[:]], outs=[cc_out_d[:]],
            )

            # g = inv * relu(h) for the final apply; overlaps the collective
            for c4 in range(NCH):
                nc.vector.tensor_scalar(
                    g_bf[:, c4, :], r_bf[:, c4, :], inv[:, c4:c4 + 1], None,
                    ALU.mult,
                )

            allst = small.tile([N_CORES, 2 * C], F32R)
            nc.sync.dma_start(allst[:], cc_out_d[:])

            # ---- reduce over ranks (PE), s,t math in [128,8] layout ----
            with tc.tile_pool(name="rpsum", bufs=1, space="PSUM") as rpsum:
                red = rpsum.tile([1, 2 * C], F32)
                for q in range(4):
                    sl = slice(q * 512, (q + 1) * 512)
                    nc.tensor.matmul(
                        red[:, sl], col32[0:8, :], allst[:, sl],
                        start=True, stop=True,
                    )
                red_sb = small.tile([1, 2 * C], F32)
                nc.vector.tensor_copy(red_sb[:, 0:C], red[:, 0:C])
                nc.scalar.copy(red_sb[:, C:2 * C], red[:, C:2 * C])

            # mu / E2 in [128, 8] block layout (c = 8p + j)
            muE = small.tile([128, 16], F32)
            nc.sync.dma_start(muE[:, 0:8], red_sb[:, 0:C])
            nc.sync.dma_start(muE[:, 8:16], red_sb[:, C:2 * C])

            tmp = small.tile([128, 8], F32)
            var = small.tile([128, 8], F32)
            s_blk = small.tile([128, 8], F32R)
            t_blk = small.tile([128, 8], F32R)
            nc.vector.tensor_tensor(tmp[:], muE[:, 0:8], muE[:, 0:8], ALU.mult)
            nc.vector.tensor_tensor(var[:], muE[:, 8:16], tmp[:], ALU.subtract)
            nc.scalar.activation(var[:], var[:], AF.Sqrt, bias=eps5[:])
            nc.vector.reciprocal(tmp[:], var[:])
            nc.vector.tensor_tensor(s_blk[:], tmp[:], gb_sb[:, 0:8], ALU.mult)
            nc.vector.tensor_tensor(tmp[:], s_blk[:], muE[:, 0:8], ALU.mult)
            nc.vector.tensor_tensor(t_blk[:], gb_sb[:, 8:16], tmp[:], ALU.subtract)

            srow = small.tile([1, C], F32R)
            trow = small.tile([1, C], F32R)
            nc.sync.dma_start(srow[:], s_blk[:])
            nc.sync.dma_start(trow[:], t_blk[:])

            S_bf = work.tile([128, C], BF16)
            T_bf = work.tile([128, C], BF16)
            with tc.tile_pool(name="bcpsum", bufs=1, space="PSUM") as bcpsum:
                Sp = bcpsum.tile([128, C], F32)
                Tp = bcpsum.tile([128, C], F32)
                for half in range(2):
                    sl = slice(half * 512, (half + 1) * 512)
                    nc.tensor.matmul(Sp[:, sl], row32[:], srow[:, sl],
                                     start=True, stop=True)
                    nc.tensor.matmul(Tp[:, sl], row32[:], trow[:, sl],
                                     start=True, stop=True)
                nc.scalar.copy(S_bf[:, 0:512], Sp[:, 0:512])
                nc.vector.tensor_copy(S_bf[:, 512:C], Sp[:, 512:C])
                nc.scalar.copy(T_bf[:, 0:512], Tp[:, 0:512])
                nc.vector.tensor_copy(T_bf[:, 512:C], Tp[:, 512:C])

            # ---- out = g*S + T ----
            for c4 in range(NCH):
                gs = applyp.tile([128, C], BF16)
                nc.vector.tensor_tensor(gs[:], g_bf[:, c4, :], S_bf[:], ALU.mult)
                ot = outp.tile([128, C], BF16)
                nc.vector.tensor_tensor(ot[:], gs[:], T_bf[:], ALU.add)
                # SWDGE casts bf16 -> f32 during the store
                nc.gpsimd.dma_start(out_d[c4 * 128:(c4 + 1) * 128, :], ot[:])

    _spread_waits(nc)
    return nc


_NC = None


def _host_prep(x, idx1, idx2, idx3, W_x_w, W_x_b, W_n_w, W_n_b, bn_gamma, bn_beta):
    x = np.ascontiguousarray(np.asarray(x, dtype=np.float32))
    idxs = [np.asarray(i).astype(np.int64) for i in (idx1, idx2, idx3)]
    W_x_w = np.asarray(W_x_w, dtype=np.float32)
    W_n_w = np.asarray(W_n_w, dtype=np.float32)
    W_x_b = np.asarray(W_x_b, dtype=np.float32)
    W_n_b = np.asarray(W_n_b, dtype=np.float32)
    bn_gamma = np.asarray(bn_gamma, dtype=np.float32)
    bn_beta = np.asarray(bn_beta, dtype=np.float32)

    # wc: [128, FCH, 2F] partition-major; rows = [WxT | WnT]; wb = [bx | bn]
    wcT = np.concatenate([W_x_w.T, W_n_w.T], axis=1)          # [F, 2F]
    wc = np.ascontiguousarray(wcT.reshape(FCH, 128, 2 * F).transpose(1, 0, 2))
    wb = np.concatenate([W_x_b, W_n_b])[None, :]              # [1, 2F]

    # A^T blocks: a[g][j, m] with columns m = [A1T | A2T | A3T] of node
    # chunk g; entries count/32 (exact in bf16).
    a = np.zeros((NCH, N, NSETS * 128), dtype=np.float32)
    for s, idx in enumerate(idxs):
        nbr = idx.reshape(N, NMAX)                            # [n, k] -> j
        cnt = np.zeros((N, N), dtype=np.float32)              # A[n, j]
        np.add.at(cnt, (np.repeat(np.arange(N), NMAX), nbr.ravel()), 1.0)
        AT = cnt.T / NMAX                                     # [j, n]
        for g in range(NCH):
            a[g, :, s * 128:(s + 1) * 128] = AT[:, g * 128:(g + 1) * 128]
    a = np.ascontiguousarray(
        a.reshape(NCH, NCH, 128, NSETS * 128).transpose(0, 2, 1, 3)
    ).astype(ml_dtypes.bfloat16)

    gb = np.concatenate(
        [bn_gamma.reshape(128, 8), bn_beta.reshape(128, 8)], axis=1
    ).astype(np.float32)

    shared = {
        "wc": wc, "wb": np.ascontiguousarray(wb),
        "a": a,
        "bnr": np.ascontiguousarray(W_n_b[None, :]).astype(ml_dtypes.bfloat16),
        "gb": np.ascontiguousarray(gb),
        "row32": np.ones((1, 128), dtype=np.float32),
        "col32": np.ones((128, 1), dtype=np.float32),
        "ident": np.eye(128, dtype=np.float32),
    }
    in_maps = []
    for c in range(N_CORES):
        m = dict(shared)
        m["x"] = np.ascontiguousarray(x[c].reshape(NCH, 128, F).transpose(1, 0, 2))
        in_maps.append(m)
    return in_maps


_NC_CACHE = {}


def kernel(**inputs):
    in_maps = _host_prep(**inputs)
    skip_wb = bool(np.all(in_maps[0]["wb"] == 0.0))
    skip_bnb = not np.any(np.asarray(in_maps[0]["bnr"], dtype=np.float32))
    key = (skip_wb, skip_bnb)
    if key not in _NC_CACHE:
        _NC_CACHE[key] = _build_module(skip_wb=skip_wb, skip_bnb=skip_bnb)
    _NC = _NC_CACHE[key]

    kwargs = {}
    trace_dir = os.environ.get("BASSK_TRACE")
    if trace_dir:
        try:
            import axon_profile_hook
            axon_profile_hook.install()
            os.makedirs(trace_dir, exist_ok=True)
            kwargs = {"trace": True, "tmpdir": trace_dir}
        except Exception as e:
            print(f"trace setup failed: {e}", file=sys.stderr)

    res = run_bass_kernel_spmd(_NC, in_maps, core_ids=list(range(N_CORES)), **kwargs)
    if trace_dir and res.exec_time_ns is not None:
        print(f"HW exec time: {res.exec_time_ns} ns")
    out = np.stack([res.results[c]["out"] for c in range(N_CORES)], axis=0)
    return out.astype(np.float32)


# revision 22
# speedup vs baseline: 1.2451x; 1.2451x over previous
"""BatchedGraphSAGEMean1Temporal Trainium2 kernel.

Strategy (8 NeuronCores, data-parallel over batch B=8, one graph/core):

The three neighbor-mean gathers share indices across the batch, so each
gather+mean is a fixed linear operator A_k [N,N] (A_k[n,j] = multiplicity
of j among n's 32 neighbors, / 32).  Since the op is linear,
    h_k = neib_mean_k(x) @ Wn^T + bn = A_k @ (x @ Wn^T) + bn,
so Wn is applied once and the three gathers become three dense matmuls
with host-prebuilt A_k (exact in bf16: entries are small-int/32).

Per-core pipeline:
  - PE-transpose x -> xT, then [h_self | y] = [x|1] @ [[WxT|WnT];[bx|bn]]
    as fp32r matmuls (full PE rate).
  - h_k = A_cat @ y + bn as bf16 matmuls (A exact, y rounded to bf16).
  - L2 norm: ACT Square with accum_out -> ss, inv = 1/sqrt(ss),
    g = max(inv*h, 0) fused on DVE.
  - BatchNorm stats: ones-column matmuls give per-channel sum/sumsq rows;
    AllGather(8KB) across the 8 cores; reduce + scale/shift math in a
    [128,8] layout; broadcast s,t via K=1 matmuls; out = g*S + T.
"""

import os
import sys

if "/opt/trn_rl_repo" not in sys.path:
    sys.path.insert(0, "/opt/trn_rl_repo")

import numpy as np
import ml_dtypes

import concourse.bass as bass
import concourse.mybir as mybir
import concourse.tile as tile
from concourse.bass_utils import run_bass_kernel_spmd
from concourse.vector_clock import ScopedClock

BF16 = mybir.dt.bfloat16
F32 = mybir.dt.float32
F32R = mybir.dt.float32r
AF = mybir.ActivationFunctionType
ALU = mybir.AluOpType

N_CORES = 8
B, N, F, C = 8, 512, 256, 1024
NMAX = 32
NCH = N // 128          # 4 node chunks
FCH = F // 128          # 2 feature chunks
NSETS = 3
BN_EPS = 1e-5
WARMUP_MMS = 8


# ---------------------------------------------------------------------------
# Workaround: walrus rejects >2 sync waits on one ctrl instruction; Tile's
# kernel-tail drain carries one wait per completion sem.  Spread them over
# nofuse NOPs (SP executes serially, so wait-then-drain is equivalent).
def _spread_drain_and_barrier(self, tick_clock, wait_clock):
    nc = self.nc
    probe = nc.sync.nop(nofuse=True)
    wait_clock.add_sem_waits(probe.ins, ScopedClock({None: tick_clock.global_clock}))
    si = probe.ins.sync_info
    waits = list(si.on_wait or []) if si is not None else []
    if si is not None:
        probe.ins.sync_info = mybir.SyncInfo(on_wait=waits[:1], on_update=si.on_update)
    for w in waits[1:]:
        n = nc.sync.nop(nofuse=True)
        n.ins.sync_info = mybir.SyncInfo(on_wait=[w], on_update=[])
    nc.sync.drain()
    nc.all_engine_barrier()
    assert self.sems is not None
    popped = nc._tile_sem_poison_stack.pop()
    assert popped is self._sem_poison
    nc.clear_and_free_semaphores(list(self.sems.allocated().values()))
    nc.all_engine_barrier()


tile.TileContext._drain_and_barrier = _spread_drain_and_barrier


def _spread_waits(nc, limit=1):
    """Move excess per-instruction sem waits onto preceding same-engine NOPs
    (walrus caps sync waits per instruction; engines execute serially so
    wait-then-op is equivalent)."""
    cnt = 0
    for fn in nc.m.functions:
        for bb in fn.blocks:
            il = bb.instructions
            new = []
            for inst in il:
                si = getattr(inst, "sync_info", None)
                waits = list(si.on_wait or []) if si is not None else []
                if len(waits) > limit:
                    keep = waits[:limit]
                    extra = waits[limit:]
                    for i in range(0, len(extra), limit):
                        nop = mybir.InstNoOp(name=f"wsplit-{cnt}", ins=[], outs=[])
                        cnt += 1
                        nop.engine = inst.engine
                        nop.sync_info = mybir.SyncInfo(
                            on_wait=extra[i:i + limit], on_update=[]
                        )
                        new.append(nop)
                    inst.sync_info = mybir.SyncInfo(
                        on_wait=keep, on_update=si.on_update
                    )
                new.append(inst)
            if len(new) != len(il):
                il[:] = new
    return cnt
# ---------------------------------------------------------------------------


def _build_module(skip_wb=False, skip_bnb=False):
    nc = bass.Bass("TRN2", target_bir_lowering=False, debug=False,
                   num_devices=N_CORES)

    # ---- DRAM I/O ----
    x_d = nc.dram_tensor("x", [128, NCH, F], F32R, kind="ExternalInput").ap()
    wc_d = nc.dram_tensor("wc", [128, FCH, 2 * F], F32R, kind="ExternalInput").ap()
    wb_d = nc.dram_tensor("wb", [1, 2 * F], F32R, kind="ExternalInput").ap()
    a_d = nc.dram_tensor("a", [NCH, 128, NCH, NSETS * 128], BF16,
                         kind="ExternalInput").ap()
    bnr_d = nc.dram_tensor("bnr", [1, F], BF16, kind="ExternalInput").ap()
    gb_d = nc.dram_tensor("gb", [128, 16], F32, kind="ExternalInput").ap()
    ident_d = nc.dram_tensor("ident", [128, 128], F32R, kind="ExternalInput").ap()
    row32_d = nc.dram_tensor("row32", [1, 128], F32R, kind="ExternalInput").ap()
    col32_d = nc.dram_tensor("col32", [128, 1], F32R, kind="ExternalInput").ap()
    out_d = nc.dram_tensor("out", [N, C], F32, kind="ExternalOutput").ap()

    cc_in_d = nc.dram_tensor("cc_in", [1, 2 * C], F32R)
    cc_out_d = nc.dram_tensor("cc_out", [N_CORES, 2 * C], F32R,
                              addr_space="Shared")

    with tile.TileContext(nc) as tc:
        with (
            tc.tile_pool(name="const", bufs=1) as constp,
            tc.tile_pool(name="inp", bufs=1) as inp,
            tc.tile_pool(name="work", bufs=1) as work,
            tc.tile_pool(name="sq", bufs=3) as sqp,
            tc.tile_pool(name="apply", bufs=2) as applyp,
            tc.tile_pool(name="outp", bufs=2) as outp,
            tc.tile_pool(name="small", bufs=1) as small,
        ):
            # ---- input DMAs: critical tensors first, split across the
            # SP and ACT HWDGE queues ----
            x_sb = inp.tile([128, NCH, F], F32R)
            nc.sync.dma_start(x_sb[:], x_d[:])
            ident = constp.tile([128, 128], F32R)
            nc.sync.dma_start(ident[:], ident_d[:])
            wc_sb = inp.tile([128, FCH, 2 * F], F32R)
            nc.sync.dma_start(wc_sb[:], wc_d[:])
            a_sb = inp.tile([128, NCH * NCH, NSETS * 128], BF16)
            for g in range(NCH):
                deng = nc.sync if g % 2 == 0 else nc.scalar
                deng.dma_start(a_sb[:, g * NCH:(g + 1) * NCH, :], a_d[g])
            wb_sb = inp.tile([1, 2 * F], F32R)
            nc.scalar.dma_start(wb_sb[:], wb_d[:])
            bnr_sb = inp.tile([1, F], BF16)
            nc.scalar.dma_start(bnr_sb[:], bnr_d[:])
            gb_sb = constp.tile([128, 16], F32)     # gamma blocks | beta blocks
            nc.scalar.dma_start(gb_sb[:], gb_d[:])
            row32 = constp.tile([1, 128], F32R)
            nc.scalar.dma_start(row32[:], row32_d[:])
            col32 = constp.tile([128, 1], F32R)
            nc.scalar.dma_start(col32[:], col32_d[:])
            row16 = constp.tile([1, 128], BF16)
            nc.vector.memset(row16[:], 1.0)
            eps5 = constp.tile([128, 1], F32)
            nc.vector.memset(eps5[:], BN_EPS)
            eps24 = constp.tile([128, 1], F32)
            nc.vector.memset(eps24[:], 1e-24)

            wsrc = constp.tile([128, 512], BF16)
            nc.vector.memset(wsrc[:], 0.0)
            wsrc = constp.tile([128, 512], BF16)
            nc.vector.memset(wsrc[:], 0.0)
            xT_sb = work.tile([128, FCH, N], F32R)
            y_bf = work.tile([128, NCH, F], BF16)
            r_bf = work.tile([128, NCH, C], BF16)
            rsq_bf = work.tile([128, NCH, C], BF16)
            g_bf = work.tile([128, NCH, C], BF16)
            ss4 = small.tile([128, 2 * NCH], F32)   # per-piece sumsq accums
            ss = small.tile([128, NCH], F32)
            std = small.tile([128, NCH], F32)
            inv = small.tile([128, NCH], F32)
            inv2 = small.tile([128, NCH], F32)
            invc = small.tile([128, 2 * NCH], BF16)

            def keep_warm(pool, n=1):
                for _ in range(n):
                    pbd = pool.tile([128, NSETS * F], F32, tag="pb")
                    nc.tensor.matmul(pbd[:, 0:F], wsrc[:, 0:128], wsrc[:, 0:F],
                                     start=True, stop=True)

            def keep_warm(pool, n=1):
                for _ in range(n):
                    pbd = pool.tile([128, NSETS * F], F32, tag="pb")
                    nc.tensor.matmul(pbd[:, 0:F], wsrc[:, 0:128], wsrc[:, 0:F],
                                     start=True, stop=True)

            def evac_piece(c4, pap, lo, hi, acc, on_act):
                """Fused PSUM evacuation: r = relu(h), sq+accum for L2."""
                dst = r_bf[:, c4, lo:hi]
                if on_act:
                    nc.scalar.activation(dst, pap, AF.Relu)
                else:
                    nc.vector.tensor_scalar(dst, pap, 0.0, None, ALU.max)
                sqt = sqp.tile([128, NSETS * F], F32, tag="sqt")
                nc.scalar.activation(
                    sqt[:, 0:hi - lo], pap, AF.Square,
                    accum_out=ss4[:, acc:acc + 1],
                )

            def chunk_stats(c4, spsum_tiles):
                """Per-chunk: ss -> inv -> stat-matmul columns -> stat MMs."""
                smu, se2 = spsum_tiles
                nc.vector.tensor_reduce(
                    ss[:, c4:c4 + 1], ss4[:, 2 * c4:2 * (c4 + 1)],
                    axis=mybir.AxisListType.X, op=ALU.add,
                )
                nc.scalar.activation(
                    std[:, c4:c4 + 1], ss[:, c4:c4 + 1], AF.Sqrt, bias=eps24[:])
                nc.vector.reciprocal(inv[:, c4:c4 + 1], std[:, c4:c4 + 1])
                nc.vector.tensor_tensor(
                    inv2[:, c4:c4 + 1], inv[:, c4:c4 + 1], inv[:, c4:c4 + 1],
                    ALU.mult)
                nc.vector.tensor_scalar(
                    invc[:, c4:c4 + 1], inv[:, c4:c4 + 1],
                    1.0 / (B * N), None, ALU.mult)
                nc.vector.tensor_scalar(
                    invc[:, NCH + c4:NCH + c4 + 1], inv2[:, c4:c4 + 1],
                    1.0 / (B * N), None, ALU.mult)
                for half in range(2):
                    sl = slice(half * 512, (half + 1) * 512)
                    nc.tensor.matmul(
                        smu[:, sl], invc[:, c4:c4 + 1], r_bf[:, c4, sl],
                        start=(c4 == 0), stop=(c4 == NCH - 1),
                    )
                    nc.tensor.matmul(
                        se2[:, sl], invc[:, NCH + c4:NCH + c4 + 1],
                        rsq_bf[:, c4, sl],
                        start=(c4 == 0), stop=(c4 == NCH - 1),
                    )

            spsum_cm = tc.tile_pool(name="spsum", bufs=1, space="PSUM")
            spsum = spsum_cm.__enter__()
            with tc.tile_pool(name="tpsum", bufs=2, space="PSUM") as tpsum:
                # ---- PE warmup burst (HAM un-throttle during input DMAs) ----
                for w in range(WARMUP_MMS):
                    wp = tpsum.tile([128, 512], F32, tag="apsum")
                    nc.tensor.matmul(wp[:], wsrc[:, 0:128], wsrc[:],
                                     start=True, stop=True)

                # ---- transpose x (PE) ----
                for c4 in range(NCH):
                    for fc in range(FCH):
                        pt = tpsum.tile([128, 128], F32R)
                        nc.tensor.transpose(
                            pt[:], x_sb[:, c4, fc * 128:(fc + 1) * 128], ident[:]
                        )
                        nc.vector.tensor_copy(
                            xT_sb[:, fc, c4 * 128:(c4 + 1) * 128], pt[:]
                        )

                smu = spsum.tile([1, C], F32)
                se2 = spsum.tile([1, C], F32)

                # ---- MM-A: [h_self | y] = [x|1] @ [[WxT|WnT];[bx|bn]] ----
                pa_tiles = []
                for c4 in range(NCH):
                    pa = tpsum.tile([128, 2 * F], F32, tag="apsum")
                    for kc in range(FCH):
                        nc.tensor.matmul(
                            pa[:],
                            xT_sb[:, kc, c4 * 128:(c4 + 1) * 128],
                            wc_sb[:, kc, :],
                            start=(kc == 0), stop=(skip_wb and kc == FCH - 1),
                        )
                    if not skip_wb:
                        nc.tensor.matmul(
                            pa[:], row32[:], wb_sb[:], start=False, stop=True,
                        )
                    nc.vector.tensor_copy(y_bf[:, c4, :], pa[:, F:2 * F])
                    evac_piece(c4, pa[:, 0:F], 0, F, 2 * c4, True)

            # ---- MM-B + fused epilogue, per node chunk ----
            with tc.tile_pool(name="bpsum", bufs=2, space="PSUM") as bpsum:
                keep_warm(bpsum, 2)
                for c4 in range(NCH):
                    pbc = bpsum.tile([128, NSETS * F], F32, tag="pb")
                    for s in range(NSETS):
                        sl = slice(s * F, (s + 1) * F)
                        for kc in range(NCH):
                            nc.tensor.matmul(
                                pbc[:, sl],
                                a_sb[:, c4 * NCH + kc, s * 128:(s + 1) * 128],
                                y_bf[:, kc, :],
                                start=(kc == 0),
                                stop=(skip_bnb and kc == NCH - 1),
                            )
                        if not skip_bnb:
                            nc.tensor.matmul(
                                pbc[:, sl], row16[:], bnr_sb[:],
                                start=False, stop=True,
                            )
                    evac_piece(c4, pbc[:], F, (NSETS + 1) * F, 2 * c4 + 1, False)
                    nc.vector.tensor_tensor(
                        rsq_bf[:, c4, :], r_bf[:, c4, :], r_bf[:, c4, :],
                        ALU.mult)
                    chunk_stats(c4, (smu, se2))
                    keep_warm(bpsum, 1)
                    keep_warm(bpsum, 1)

                stat_sb = small.tile([1, 2 * C], F32R)
                nc.vector.tensor_copy(stat_sb[:, 0:C], smu[:])
                nc.scalar.copy(stat_sb[:, C:2 * C], se2[:])
            spsum_cm.__exit__(None, None, None)

            # ---- AllGather local stats across the 8 cores ----
            nc.sync.dma_start(cc_in_d[:], stat_sb[:])
            nc.gpsimd.collective_compute(
                "AllGather", ALU.bypass,
                replica_groups=[list(range(N_CORES))],
                ins=[cc_in_d[:]], outs=[cc_out_d[:]],
            )

            # g = inv * relu(h) for the final apply; overlaps the collective
            for c4 in range(NCH):
                nc.vector.tensor_scalar(
                    g_bf[:, c4, :], r_bf[:, c4, :], inv[:, c4:c4 + 1], None,
                    ALU.mult,
                )

            allst = small.tile([N_CORES, 2 * C], F32R)
            nc.sync.dma_start(allst[:], cc_out_d[:])

            # ---- reduce over ranks (PE), s,t math in [128,8] layout ----
            with tc.tile_pool(name="rpsum", bufs=1, space="PSUM") as rpsum:
                red = rpsum.tile([1, 2 * C], F32)
                for q in range(4):
                    sl = slice(q * 512, (q + 1) * 512)
                    nc.tensor.matmul(
                        red[:, sl], col32[0:8, :], allst[:, sl],
                        start=True, stop=True,
                    )
                red_sb = small.tile([1, 2 * C], F32)
                nc.vector.tensor_copy(red_sb[:, 0:C], red[:, 0:C])
                nc.scalar.copy(red_sb[:, C:2 * C], red[:, C:2 * C])

            # mu / E2 in [128, 8] block layout (c = 8p + j)
            muE = small.tile([128, 16], F32)
            nc.sync.dma_start(muE[:, 0:8], red_sb[:, 0:C])
            nc.sync.dma_start(muE[:, 8:16], red_sb[:, C:2 * C])

            tmp = small.tile([128, 8], F32)
            var = small.tile([128, 8], F32)
            s_blk = small.tile([128, 8], F32R)
            t_blk = small.tile([128, 8], F32R)
            nc.vector.tensor_tensor(tmp[:], muE[:, 0:8], muE[:, 0:8], ALU.mult)
            nc.vector.tensor_tensor(var[:], muE[:, 8:16], tmp[:], ALU.subtract)
            nc.scalar.activation(var[:], var[:], AF.Sqrt, bias=eps5[:])
            nc.vector.reciprocal(tmp[:], var[:])
            nc.vector.tensor_tensor(s_blk[:], tmp[:], gb_sb[:, 0:8], ALU.mult)
            nc.vector.tensor_tensor(tmp[:], s_blk[:], muE[:, 0:8], ALU.mult)
            nc.vector.tensor_tensor(t_blk[:], gb_sb[:, 8:16], tmp[:], ALU.subtract)

            srow = small.tile([1, C], F32R)
            trow = small.tile([1, C], F32R)
            nc.sync.dma_start(srow[:], s_blk[:])
            nc.sync.dma_start(trow[:], t_blk[:])

            S_bf = work.tile([128, C], BF16)
            T_bf = work.tile([128, C], BF16)
            with tc.tile_pool(name="bcpsum", bufs=1, space="PSUM") as bcpsum:
                Sp = bcpsum.tile([128, C], F32)
                Tp = bcpsum.tile([128, C], F32)
                for half in range(2):
                    sl = slice(half * 512, (half + 1) * 512)
                    nc.tensor.matmul(Sp[:, sl], row32[:], srow[:, sl],
                                     start=True, stop=True)
                    nc.tensor.matmul(Tp[:, sl], row32[:], trow[:, sl],
                                     start=True, stop=True)
                nc.scalar.copy(S_bf[:, 0:512], Sp[:, 0:512])
                nc.vector.tensor_copy(S_bf[:, 512:C], Sp[:, 512:C])
                nc.scalar.copy(T_bf[:, 0:512], Tp[:, 0:512])
                nc.vector.tensor_copy(T_bf[:, 512:C], Tp[:, 512:C])

            # ---- out = g*S + T ----
            for c4 in range(NCH):
                gs = applyp.tile([128, C], BF16)
                nc.vector.tensor_tensor(gs[:], g_bf[:, c4, :], S_bf[:], ALU.mult)
                ot = outp.tile([128, C], BF16)
                nc.vector.tensor_tensor(ot[:], gs[:], T_bf[:], ALU.add)
                # SWDGE casts bf16 -> f32 during the store
                nc.gpsimd.dma_start(out_d[c4 * 128:(c4 + 1) * 128, :], ot[:])

    _spread_waits(nc)
    return nc


_NC = None


def _host_prep(x, idx1, idx2, idx3, W_x_w, W_x_b, W_n_w, W_n_b, bn_gamma, bn_beta):
    x = np.ascontiguousarray(np.asarray(x, dtype=np.float32))
    idxs = [np.asarray(i).astype(np.int64) for i in (idx1, idx2, idx3)]
    W_x_w = np.asarray(W_x_w, dtype=np.float32)
    W_n_w = np.asarray(W_n_w, dtype=np.float32)
    W_x_b = np.asarray(W_x_b, dtype=np.float32)
    W_n_b = np.asarray(W_n_b, dtype=np.float32)
    bn_gamma = np.asarray(bn_gamma, dtype=np.float32)
    bn_beta = np.asarray(bn_beta, dtype=np.float32)

    # wc: [128, FCH, 2F] partition-major; rows = [WxT | WnT]; wb = [bx | bn]
    wcT = np.concatenate([W_x_w.T, W_n_w.T], axis=1)          # [F, 2F]
    wc = np.ascontiguousarray(wcT.reshape(FCH, 128, 2 * F).transpose(1, 0, 2))
    wb = np.concatenate([W_x_b, W_n_b])[None, :]              # [1, 2F]

    # A^T blocks: a[g][j, m] with columns m = [A1T | A2T | A3T] of node
    # chunk g; entries count/32 (exact in bf16).
    a = np.zeros((NCH, N, NSETS * 128), dtype=np.float32)
    for s, idx in enumerate(idxs):
        nbr = idx.reshape(N, NMAX)                            # [n, k] -> j
        cnt = np.zeros((N, N), dtype=np.float32)              # A[n, j]
        np.add.at(cnt, (np.repeat(np.arange(N), NMAX), nbr.ravel()), 1.0)
        AT = cnt.T / NMAX                                     # [j, n]
        for g in range(NCH):
            a[g, :, s * 128:(s + 1) * 128] = AT[:, g * 128:(g + 1) * 128]
    a = np.ascontiguousarray(
        a.reshape(NCH, NCH, 128, NSETS * 128).transpose(0, 2, 1, 3)
    ).astype(ml_dtypes.bfloat16)

    gb = np.concatenate(
        [bn_gamma.reshape(128, 8), bn_beta.reshape(128, 8)], axis=1
    ).astype(np.float32)

    shared = {
        "wc": wc, "wb": np.ascontiguousarray(wb),
        "a": a,
        "bnr": np.ascontiguousarray(W_n_b[None, :]).astype(ml_dtypes.bfloat16),
        "gb": np.ascontiguousarray(gb),
        "row32": np.ones((1, 128), dtype=np.float32),
        "col32": np.ones((128, 1), dtype=np.float32),
        "ident": np.eye(128, dtype=np.float32),
    }
    in_maps = []
    for c in range(N_CORES):
        m = dict(shared)
        m["x"] = np.ascontiguousarray(x[c].reshape(NCH, 128, F).transpose(1, 0, 2))
        in_maps.append(m)
    return in_maps


_NC_CACHE = {}


def kernel(**inputs):
    in_maps = _host_prep(**inputs)
    skip_wb = bool(np.all(in_maps[0]["wb"] == 0.0))
    skip_bnb = not np.any(np.asarray(in_maps[0]["bnr"], dtype=np.float32))
    key = (skip_wb, skip_bnb)
    if key not in _NC_CACHE:
        _NC_CACHE[key] = _build_module(skip_wb=skip_wb, skip_bnb=skip_bnb)
    _NC = _NC_CACHE[key]

    kwargs = {}
    trace_dir = os.environ.get("BASSK_TRACE")
    if trace_dir:
        try:
            import axon_profile_hook
            axon_profile_hook.install()
            os.makedirs(trace_dir, exist_ok=True)
            kwargs = {"trace": True, "tmpdir": trace_dir}
        except Exception as e:
            print(f"trace setup failed: {e}", file=sys.stderr)

    res = run_bass_kernel_spmd(_NC, in_maps, core_ids=list(range(N_CORES)), **kwargs)
    if trace_dir and res.exec_time_ns is not None:
        print(f"HW exec time: {res.exec_time_ns} ns")
    out = np.stack([res.results[c]["out"] for c in range(N_CORES)], axis=0)
    return out.astype(np.float32)
